# revision 12
# baseline (speedup 1.0000x reference)
"""Causal self-attention (B=2, T=2048, C=1024, H=16, hd=64) on 8 TRN2 cores.

Sharding: 2-way batch x 4-way head-group (4 heads/core). Each core
computes qkv for its heads, causal attention, and a row-parallel partial
of the out-projection; the host sums the 4 partials per batch element.

v2: all matmul operands bf16 (host-converted inputs), exact causal
column starts (no 256-col floor), causal mask via DVE multiply with a
precomputed 128x128 triangle tile (gpsimd freed for weight DMA), and
AV matmuls emitted LAG score-blocks behind their scores so the in-order
PE queue never stalls on ScalarE's exp latency.

Device schedule (per core, software-pipelined over 4 query windows):
  A(w): qk^T window (wqk^T x^T -> [512 rows, 512 t]) + v blocks
  B(w,pair): per kj block: S^T = k^T q (two 64-row matmuls, PE row-tile
        concurrent), exp (ScalarE, exact causal-shrunk columns), tri
        mask (DVE mul, diag blocks only), then LAG blocks later
        AV += v_aug^T expS^T (ones column gives softmax denom, row 64)
  C(w): denom broadcast (K=1 matmul), reciprocal, scale; odd heads
        DMA-shifted to partitions 64-127 of the pair tile
  D(w): out partial [512, 1024] = sum_pairs pair^T W_out rows (K=128)
"""
import numpy as np
import ml_dtypes

import concourse.bass as bass
import concourse.tile as tile
from concourse import mybir
from concourse.bass_utils import run_bass_kernel_spmd

B, T, C = 2, 2048, 1024
H, HD = 16, 64
G = 4            # heads per core
N_CORES = 8
F32 = mybir.dt.float32
BF16 = mybir.dt.bfloat16
TQ = 512         # query window (moving free dim)
TK = 128         # key block (psum partition)
NQ = T // TQ     # 4 query windows
NK = T // TK     # 16 key blocks
CCH = C // 128   # 8 contraction chunks
LAG = 3          # AV trails scores by LAG blocks (covers exp latency)


def _legalize_single_wait(nc):
    """This walrus build rejects >1 sync wait per instruction. Split
    multi-wait instructions into single-wait NoOp carriers on the same
    engine queue (in-order execution keeps semantics identical)."""
    n = 0
    for f in nc.m.functions:
        for blk in f.blocks:
            insts = list(blk.instructions)
            new = []
            changed = False
            for inst in insts:
                si = inst.sync_info
                if si is not None and len(si.on_wait) > 1:
                    waits = list(si.on_wait)
                    for j, w in enumerate(waits[:-1]):
                        new.append(mybir.InstNoOp(
                            name=f"{inst.name}-sw{j}",
                            engine=inst.engine,
                            sync_info=mybir.SyncInfo(on_wait=[w], on_update=[]),
                            bass_nofuse=True,
                        ))
                    inst.sync_info = mybir.SyncInfo(
                        on_wait=[waits[-1]], on_update=list(si.on_update))
                    changed = True
                    n += 1
                new.append(inst)
            if changed:
                blk.instructions = new
    return n


def build_nc(legalize=True, reps=1, loop_reps=None, phases="ABCD"):
    nc = bass.Bass("TRN2", target_bir_lowering=False, debug=False,
                   num_devices=N_CORES)
    xT = nc.dram_tensor("xT", [C, T], BF16, kind="ExternalInput").ap()
    wqk = nc.dram_tensor("wqk", [C, 2 * G * HD], BF16, kind="ExternalInput").ap()
    wv = nc.dram_tensor("wv", [C, G * HD], BF16, kind="ExternalInput").ap()
    wout = nc.dram_tensor("wout", [G * HD, C], BF16, kind="ExternalInput").ap()
    trimask = nc.dram_tensor("trimask", [TK, TK], BF16, kind="ExternalInput").ap()
    out = nc.dram_tensor("out", [T, C], BF16, kind="ExternalOutput").ap()

    with tile.TileContext(nc) as tc, \
         nc.allow_low_precision(reason="bf16 kernel, 2e-2 rel-err budget"):
        with tc.tile_pool(name="weights", bufs=1) as wpool, \
             tc.tile_pool(name="qkT", bufs=1) as qkpool, \
             tc.tile_pool(name="vsb", bufs=1) as vpool, \
             tc.tile_pool(name="outh", bufs=1) as opool, \
             tc.tile_pool(name="pairs", bufs=1) as ppool, \
             tc.tile_pool(name="xt", bufs=2) as xtp, \
             tc.tile_pool(name="expS", bufs=LAG + 3) as expp, \
             tc.tile_pool(name="div", bufs=3) as dpool, \
             tc.tile_pool(name="osb", bufs=1) as osb, \
             tc.tile_pool(name="mm_ps", bufs=2, space="PSUM") as mmps, \
             tc.tile_pool(name="sc_ps", bufs=2, space="PSUM") as scps, \
             tc.tile_pool(name="av_ps", bufs=1, space="PSUM") as avps:

            wqk_sb = wpool.tile([128, CCH, 512], BF16)
            wv_sb = wpool.tile([128, CCH, 256], BF16)
            wout_sb = wpool.tile([128, 2, 1024], BF16)
            # lhsT for the K=1 denominator-broadcast matmul must share the
            # rhs's base partition (64 = outhT's denominator row)
            ones_sb = wpool.tile([128, 64], BF16)
            tri_sb = wpool.tile([TK, TK], BF16)
            # weights stream on the (initially idle) gpsimd queue, split in
            # chunks so the first matmuls start while the rest arrives; xt
            # loads use the sync queue in parallel
            nc.gpsimd.dma_start(out=tri_sb, in_=trimask)
            for half in range(2):
                nc.gpsimd.dma_start(
                    out=wqk_sb[:, 4 * half:4 * half + 4, :],
                    in_=wqk.rearrange("(c p) n -> p c n", p=128)
                        [:, 4 * half:4 * half + 4, :])
            nc.gpsimd.dma_start(
                out=wv_sb, in_=wv.rearrange("(c p) n -> p c n", p=128))
            nc.gpsimd.dma_start(
                out=wout_sb, in_=wout.rearrange("(c p) n -> p c n", p=128))
            nc.vector.memset(ones_sb, 1.0)

            # qkT[:, nt, :]: nt 0-1 = q^T of heads (0,1),(2,3); nt 2-3 = k^T
            qkT = qkpool.tile([128, 4, T], BF16)
            # v_sb[:, kj, h, 0:64] = v, [..., 64] = 1.0 (only the ones
            # column is memset; the rest is overwritten by A-phase copies)
            v_sb = vpool.tile([128, NK, G, HD + 1], BF16)
            nc.vector.memset(v_sb[:, :, :, HD:HD + 1], 1.0)
            # outhT[h]: [65, T]; rows 0-63 = (expS @ v)^T, row 64 = denom
            outhT = [opool.tile([HD + 1, T], BF16, tag=f"outh{h}", name=f"outhT{h}")
                     for h in range(G)]
            # pair_sb[p]: [128, T] = scaled head 2p (rows 0-63) / 2p+1 (64-127)
            pair_sb = [ppool.tile([128, T], BF16, tag=f"pair{p}", name=f"pair{p}")
                       for p in range(2)]

            def emit_A(w):
                """qk^T window + v blocks, as one unit per matmul group."""
                tsl = slice(w * TQ, (w + 1) * TQ)
                xt = xtp.tile([128, CCH, TQ], BF16, name=f"xt{w}", tag="xt")
                for half in range(2):
                    nc.sync.dma_start(
                        out=xt[:, 4 * half:4 * half + 4, :],
                        in_=xT.rearrange("(c p) t -> p c t", p=128)
                            [:, 4 * half:4 * half + 4, tsl])
                units = []

                def qk_unit(nt):
                    ps = mmps.tile([128, TQ], F32, tag="mm", name=f"qk{w}_{nt}")
                    for cc in range(CCH):
                        nc.tensor.matmul(
                            ps, lhsT=wqk_sb[:, cc, nt * 128:(nt + 1) * 128],
                            rhs=xt[:, cc, :], start=(cc == 0), stop=(cc == CCH - 1))
                    nc.vector.tensor_copy(out=qkT[:, nt, tsl], in_=ps)

                def v_unit(sub):
                    kj = w * (TQ // TK) + sub
                    psv = mmps.tile([128, G * HD], F32, tag="mm", name=f"v{w}_{sub}")
                    for cc in range(CCH):
                        nc.tensor.matmul(
                            psv, lhsT=xt[:, cc, sub * TK:(sub + 1) * TK],
                            rhs=wv_sb[:, cc, :], start=(cc == 0), stop=(cc == CCH - 1))
                    nc.vector.tensor_copy(
                        out=v_sb[:, kj, :, 0:HD],
                        in_=psv.rearrange("p (h d) -> p h d", h=G))

                for nt in range(4):
                    units.append(lambda nt=nt: qk_unit(nt))
                for sub in range(TQ // TK):
                    units.append(lambda sub=sub: v_unit(sub))
                return units

            def emit_B(w, pair):
                """Scores + exp + mask per kj; AV lagged LAG blocks behind."""
                tsl = slice(w * TQ, (w + 1) * TQ)
                kmax = (w + 1) * (TQ // TK)
                h0, h1 = 2 * pair, 2 * pair + 1
                av = [avps.tile([HD + 1, TQ], F32, tag=f"av{e}",
                                name=f"av{pair}_{w}_{e}") for e in range(2)]
                exs = {}

                def sc_unit(kj):
                    d = kj * TK - w * TQ      # >=0 on diagonal blocks
                    cs = max(d, 0)
                    sc = scps.tile([128, 2, TQ], F32, tag="sc",
                                   name=f"sc{pair}_{w}_{kj}")
                    for e, h in enumerate((h0, h1)):
                        row = 64 * (h % 2)
                        nt = h // 2
                        nc.tensor.matmul(
                            sc[:, e, cs:],
                            lhsT=qkT[row:row + 64, 2 + nt, kj * TK:(kj + 1) * TK],
                            rhs=qkT[row:row + 64, nt, w * TQ + cs:(w + 1) * TQ],
                            start=True, stop=True)
                    ex = expp.tile([128, 2, TQ], BF16, tag="ex",
                                   name=f"ex{pair}_{w}_{kj}")
                    nc.scalar.activation(
                        out=ex[:, :, cs:], in_=sc[:, :, cs:],
                        func=mybir.ActivationFunctionType.Exp,
                        scale=1.0 / np.sqrt(HD))
                    if d >= 0:
                        # noncausal entries live only in the first TK columns
                        # of the trimmed slice: col_local < key_partition
                        for e in range(2):
                            nc.vector.tensor_mul(
                                out=ex[:, e, cs:cs + TK],
                                in0=ex[:, e, cs:cs + TK], in1=tri_sb)
                    exs[kj] = ex

                def av_unit(kj):
                    cs = max(kj * TK - w * TQ, 0)
                    ex = exs[kj]
                    for e, h in enumerate((h0, h1)):
                        nc.tensor.matmul(
                            av[e][:, cs:], lhsT=v_sb[:, kj, h, :],
                            rhs=ex[:, e, cs:],
                            start=(kj == 0), stop=(kj == kmax - 1))
                    if kj == kmax - 1:
                        for e, h in enumerate((h0, h1)):
                            nc.vector.tensor_copy(
                                out=outhT[h][:, tsl], in_=av[e])

                units = []
                for kj in range(kmax):
                    units.append(lambda kj=kj: sc_unit(kj))
                    if kj >= LAG:
                        units.append(lambda kj=kj: av_unit(kj - LAG))
                for kj in range(max(kmax - LAG, 0), kmax):
                    units.append(lambda kj=kj: av_unit(kj))
                return units

            def emit_C(w, heads):
                tsl = slice(w * TQ, (w + 1) * TQ)
                units = []

                def c_unit(h):
                    # bc borrows the fast-cycling "mm" psum slots (a
                    # dedicated tag would exceed the 8-bank budget, and
                    # borrowing av slots can deadlock the static PE order
                    # against an in-flight accumulation)
                    bc = mmps.tile([64, TQ], F32, tag="mm", name=f"bc{h}_{w}")
                    nc.tensor.matmul(bc, lhsT=ones_sb[64:65, :],
                                     rhs=outhT[h][HD:HD + 1, tsl],
                                     start=True, stop=True)
                    rc = dpool.tile([64, TQ], BF16, tag="rc", name=f"rc{h}_{w}")
                    nc.vector.reciprocal(out=rc, in_=bc)
                    if h % 2 == 0:
                        nc.vector.tensor_mul(
                            out=pair_sb[h // 2][0:64, tsl],
                            in0=outhT[h][0:HD, tsl], in1=rc)
                    else:
                        tmp = dpool.tile([64, TQ], BF16, tag="shift",
                                         name=f"sh{h}_{w}")
                        nc.vector.tensor_mul(
                            out=tmp, in0=outhT[h][0:HD, tsl], in1=rc)
                        nc.sync.dma_start(
                            out=pair_sb[h // 2][64:128, tsl], in_=tmp)

                for h in heads:
                    units.append(lambda h=h: c_unit(h))
                return units

            def emit_D(w):
                units = []

                state = {}

                def d_unit(sub):
                    qt = w * (TQ // TK) + sub
                    if sub == 0:
                        state["ob"] = osb.tile([128, TQ // TK, 1024], BF16,
                                               tag="ob", name=f"ob{w}")
                    po = scps.tile([128, 2, 512], F32, tag="sc", name=f"po{qt}")
                    for p in range(2):
                        for ct in range(2):
                            nc.tensor.matmul(
                                po[:, ct, :],
                                lhsT=pair_sb[p][:, qt * 128:(qt + 1) * 128],
                                rhs=wout_sb[:, p, ct * 512:(ct + 1) * 512],
                                start=(p == 0), stop=(p == 1))
                    ob = state["ob"]
                    if w == NQ - 1:
                        nc.scalar.copy(out=ob[:, sub, :],
                                       in_=po.rearrange("p a b -> p (a b)"))
                    else:
                        nc.vector.tensor_copy(
                            out=ob[:, sub, :], in_=po.rearrange("p a b -> p (a b)"))
                    if sub == TQ // TK - 1:
                        nc.sync.dma_start(
                            out=out[w * TQ:(w + 1) * TQ, :].rearrange(
                                "(s p) c -> p s c", p=128),
                            in_=ob)

                for sub in range(TQ // TK):
                    units.append(lambda sub=sub: d_unit(sub))
                return units

            def interleave(primary, extra):
                """Spread extra units evenly among primary units."""
                out, ei = [], 0
                n, m = len(primary), len(extra)
                for i, u in enumerate(primary):
                    out.append(u)
                    while m and ei < m and ei * n < m * (i + 1):
                        out.append(extra[ei])
                        ei += 1
                out.extend(extra[ei:])
                return out

            # Software pipeline: B(w) runs interleaved with the previous
            # window's divide/out-proj and the next window's projections,
            # so the PE has matmul work while ScalarE chews on exps.
            # C for a pair starts as soon as that pair's AV is drained.
            # reps>1 repeats the whole program for wall-clock calibration.
            import contextlib
            loop_cm = tc.For_i(0, loop_reps, 1) if loop_reps else contextlib.nullcontext()
            with loop_cm:
              for _rep in range(reps):
                hasB = "B" in phases
                hasC = "C" in phases and hasB
                hasD = "D" in phases and hasC
                for u in emit_A(0):
                    u()
                for w in range(NQ):
                    b0 = emit_B(w, 0) if hasB else []
                    extra0 = (emit_C(w - 1, (2, 3)) + (emit_D(w - 1) if hasD else [])) if (w >= 1 and hasC) else []
                    for u in interleave(b0, extra0):
                        u()
                    b1 = emit_B(w, 1) if hasB else []
                    extra1 = emit_C(w, (0, 1)) if hasC else []
                    if w + 1 < NQ:
                        extra1 += emit_A(w + 1)
                    for u in interleave(b1, extra1):
                        u()
                if hasC:
                    for u in emit_C(NQ - 1, (2, 3)) + (emit_D(NQ - 1) if hasD else []):
                        u()

    if legalize:
        _legalize_single_wait(nc)
    return nc


_NC_CACHE = None


def _get_nc():
    global _NC_CACHE
    if _NC_CACHE is None:
        _NC_CACHE = build_nc()
    return _NC_CACHE


BF = ml_dtypes.bfloat16


def _shard_inputs(x, W_qkv, W_out):
    tri = np.ascontiguousarray(
        np.triu(np.ones((TK, TK), dtype=np.float32))).astype(BF)
    in_maps = []
    for b in range(B):
        xTb = np.ascontiguousarray(x[b].T).astype(BF)
        for g in range(4):
            heads = list(range(G * g, G * g + G))
            qcols = np.concatenate(
                [W_qkv[:, h * HD:(h + 1) * HD] for h in heads], axis=1)
            kcols = np.concatenate(
                [W_qkv[:, C + h * HD:C + (h + 1) * HD] for h in heads], axis=1)
            wqk = np.ascontiguousarray(
                np.concatenate([qcols, kcols], axis=1)).astype(BF)
            wv = np.ascontiguousarray(
                W_qkv[:, 2 * C + G * g * HD:2 * C + (G * g + G) * HD]).astype(BF)
            wout = np.ascontiguousarray(
                W_out[G * g * HD:(G * g + G) * HD, :]).astype(BF)
            in_maps.append({"xT": xTb, "wqk": wqk, "wv": wv, "wout": wout,
                            "trimask": tri})
    return in_maps


def kernel(x, W_qkv, W_out):
    x = np.asarray(x, dtype=np.float32)
    W_qkv = np.asarray(W_qkv, dtype=np.float32)
    W_out = np.asarray(W_out, dtype=np.float32)
    nc = _get_nc()
    in_maps = _shard_inputs(x, W_qkv, W_out)
    res = run_bass_kernel_spmd(nc, in_maps, list(range(N_CORES))).results
    out = np.zeros((B, T, C), dtype=np.float32)
    for b in range(B):
        acc = np.zeros((T, C), dtype=np.float64)
        for g in range(4):
            acc += res[4 * b + g]["out"].astype(np.float64)
        out[b] = acc.astype(np.float32)
    return out


# revision 13
# speedup vs baseline: 1.0206x; 1.0206x over previous
"""Causal self-attention (B=2, T=2048, C=1024, H=16, hd=64) on 8 TRN2 cores.

Sharding: 2-way batch x 4-way head-group (4 heads/core). Each core
computes qkv for its heads, causal attention, and a row-parallel partial
of the out-projection; the host sums the 4 partials per batch element.

v2: all matmul operands bf16 (host-converted inputs), exact causal
column starts (no 256-col floor), causal mask via DVE multiply with a
precomputed 128x128 triangle tile (gpsimd freed for weight DMA), and
AV matmuls emitted LAG score-blocks behind their scores so the in-order
PE queue never stalls on ScalarE's exp latency.

Device schedule (per core, software-pipelined over 4 query windows):
  A(w): qk^T window (wqk^T x^T -> [512 rows, 512 t]) + v blocks
  B(w,pair): per kj block: S^T = k^T q (two 64-row matmuls, PE row-tile
        concurrent), exp (ScalarE, exact causal-shrunk columns), tri
        mask (DVE mul, diag blocks only), then LAG blocks later
        AV += v_aug^T expS^T (ones column gives softmax denom, row 64)
  C(w): denom broadcast (K=1 matmul), reciprocal, scale; odd heads
        DMA-shifted to partitions 64-127 of the pair tile
  D(w): out partial [512, 1024] = sum_pairs pair^T W_out rows (K=128)
"""
import numpy as np
import ml_dtypes

import concourse.bass as bass
import concourse.tile as tile
from concourse import mybir
from concourse.bass_utils import run_bass_kernel_spmd

B, T, C = 2, 2048, 1024
H, HD = 16, 64
G = 4            # heads per core
N_CORES = 8
F32 = mybir.dt.float32
BF16 = mybir.dt.bfloat16
TQ = 512         # query window (moving free dim)
TK = 128         # key block (psum partition)
NQ = T // TQ     # 4 query windows
NK = T // TK     # 16 key blocks
CCH = C // 128   # 8 contraction chunks
LAG = 3          # AV trails scores by LAG blocks (covers exp latency)


def _legalize_single_wait(nc):
    """This walrus build rejects >1 sync wait per instruction. Split
    multi-wait instructions into single-wait NoOp carriers on the same
    engine queue (in-order execution keeps semantics identical)."""
    n = 0
    for f in nc.m.functions:
        for blk in f.blocks:
            insts = list(blk.instructions)
            new = []
            changed = False
            for inst in insts:
                si = inst.sync_info
                if si is not None and len(si.on_wait) > 1:
                    waits = list(si.on_wait)
                    for j, w in enumerate(waits[:-1]):
                        new.append(mybir.InstNoOp(
                            name=f"{inst.name}-sw{j}",
                            engine=inst.engine,
                            sync_info=mybir.SyncInfo(on_wait=[w], on_update=[]),
                            bass_nofuse=True,
                        ))
                    inst.sync_info = mybir.SyncInfo(
                        on_wait=[waits[-1]], on_update=list(si.on_update))
                    changed = True
                    n += 1
                new.append(inst)
            if changed:
                blk.instructions = new
    return n


def build_nc(legalize=True, reps=1, loop_reps=None, phases="ABCD"):
    nc = bass.Bass("TRN2", target_bir_lowering=False, debug=False,
                   num_devices=N_CORES)
    xT = nc.dram_tensor("xT", [C, T], BF16, kind="ExternalInput").ap()
    wqk = nc.dram_tensor("wqk", [C, 2 * G * HD], BF16, kind="ExternalInput").ap()
    wv = nc.dram_tensor("wv", [C, G * HD], BF16, kind="ExternalInput").ap()
    wout = nc.dram_tensor("wout", [G * HD, C], BF16, kind="ExternalInput").ap()
    trimask = nc.dram_tensor("trimask", [TK, TK], BF16, kind="ExternalInput").ap()
    out = nc.dram_tensor("out", [T, C], BF16, kind="ExternalOutput").ap()

    with tile.TileContext(nc) as tc, \
         nc.allow_low_precision(reason="bf16 kernel, 2e-2 rel-err budget"):
        with tc.tile_pool(name="weights", bufs=1) as wpool, \
             tc.tile_pool(name="qkT", bufs=1) as qkpool, \
             tc.tile_pool(name="vsb", bufs=1) as vpool, \
             tc.tile_pool(name="outh", bufs=1) as opool, \
             tc.tile_pool(name="pairs", bufs=1) as ppool, \
             tc.tile_pool(name="xt", bufs=2) as xtp, \
             tc.tile_pool(name="expS", bufs=LAG + 3) as expp, \
             tc.tile_pool(name="div", bufs=3) as dpool, \
             tc.tile_pool(name="osb", bufs=1) as osb, \
             tc.tile_pool(name="mm_ps", bufs=2, space="PSUM") as mmps, \
             tc.tile_pool(name="sc_ps", bufs=2, space="PSUM") as scps, \
             tc.tile_pool(name="av_ps", bufs=1, space="PSUM") as avps:

            wqk_sb = wpool.tile([128, CCH, 512], BF16)
            wv_sb = wpool.tile([128, CCH, 256], BF16)
            wout_sb = wpool.tile([128, 2, 1024], BF16)
            # lhsT for the K=1 denominator-broadcast matmul must share the
            # rhs's base partition (64 = outhT's denominator row)
            ones_sb = wpool.tile([128, 64], BF16)
            tri_sb = wpool.tile([TK, TK], BF16)
            # weights stream on the (initially idle) gpsimd queue, split in
            # chunks so the first matmuls start while the rest arrives; xt
            # loads use the sync queue in parallel
            nc.gpsimd.dma_start(out=tri_sb, in_=trimask)
            for half in range(2):
                nc.gpsimd.dma_start(
                    out=wqk_sb[:, 4 * half:4 * half + 4, :],
                    in_=wqk.rearrange("(c p) n -> p c n", p=128)
                        [:, 4 * half:4 * half + 4, :])
            nc.gpsimd.dma_start(
                out=wv_sb, in_=wv.rearrange("(c p) n -> p c n", p=128))
            nc.gpsimd.dma_start(
                out=wout_sb, in_=wout.rearrange("(c p) n -> p c n", p=128))
            nc.vector.memset(ones_sb, 1.0)

            # qkT[:, nt, :]: nt 0-1 = q^T of heads (0,1),(2,3); nt 2-3 = k^T
            qkT = qkpool.tile([128, 4, T], BF16)
            # v_sb[:, kj, h, 0:64] = v, [..., 64] = 1.0 (only the ones
            # column is memset; the rest is overwritten by A-phase copies)
            v_sb = vpool.tile([128, NK, G, HD + 1], BF16)
            nc.vector.memset(v_sb[:, :, :, HD:HD + 1], 1.0)
            # outhT[h]: [65, T]; rows 0-63 = (expS @ v)^T, row 64 = denom
            outhT = [opool.tile([HD + 1, T], BF16, tag=f"outh{h}", name=f"outhT{h}")
                     for h in range(G)]
            # pair_sb[p]: [128, T] = scaled head 2p (rows 0-63) / 2p+1 (64-127)
            pair_sb = [ppool.tile([128, T], BF16, tag=f"pair{p}", name=f"pair{p}")
                       for p in range(2)]

            def emit_A(w):
                """qk^T window + v blocks, as one unit per matmul group."""
                tsl = slice(w * TQ, (w + 1) * TQ)
                xt = xtp.tile([128, CCH, TQ], BF16, name=f"xt{w}", tag="xt")
                for half in range(2):
                    nc.sync.dma_start(
                        out=xt[:, 4 * half:4 * half + 4, :],
                        in_=xT.rearrange("(c p) t -> p c t", p=128)
                            [:, 4 * half:4 * half + 4, tsl])
                units = []

                def qk_unit(nt):
                    ps = mmps.tile([128, TQ], F32, tag="mm", name=f"qk{w}_{nt}")
                    for cc in range(CCH):
                        nc.tensor.matmul(
                            ps, lhsT=wqk_sb[:, cc, nt * 128:(nt + 1) * 128],
                            rhs=xt[:, cc, :], start=(cc == 0), stop=(cc == CCH - 1))
                    nc.vector.tensor_copy(out=qkT[:, nt, tsl], in_=ps)

                def v_unit(sub):
                    kj = w * (TQ // TK) + sub
                    psv = mmps.tile([128, G * HD], F32, tag="mm", name=f"v{w}_{sub}")
                    for cc in range(CCH):
                        nc.tensor.matmul(
                            psv, lhsT=xt[:, cc, sub * TK:(sub + 1) * TK],
                            rhs=wv_sb[:, cc, :], start=(cc == 0), stop=(cc == CCH - 1))
                    nc.vector.tensor_copy(
                        out=v_sb[:, kj, :, 0:HD],
                        in_=psv.rearrange("p (h d) -> p h d", h=G))

                for nt in range(4):
                    units.append(lambda nt=nt: qk_unit(nt))
                for sub in range(TQ // TK):
                    units.append(lambda sub=sub: v_unit(sub))
                return units

            def emit_B(w, pair):
                """Scores + exp + mask per kj; AV lagged LAG blocks behind."""
                tsl = slice(w * TQ, (w + 1) * TQ)
                kmax = (w + 1) * (TQ // TK)
                h0, h1 = 2 * pair, 2 * pair + 1
                av = [avps.tile([HD + 1, TQ], F32, tag=f"av{e}",
                                name=f"av{pair}_{w}_{e}") for e in range(2)]
                exs = {}

                def sc_unit(kj):
                    d = kj * TK - w * TQ      # >=0 on diagonal blocks
                    cs = max(d, 0)
                    sc = scps.tile([128, 2, TQ], F32, tag="sc",
                                   name=f"sc{pair}_{w}_{kj}")
                    for e, h in enumerate((h0, h1)):
                        row = 64 * (h % 2)
                        nt = h // 2
                        nc.tensor.matmul(
                            sc[:, e, cs:],
                            lhsT=qkT[row:row + 64, 2 + nt, kj * TK:(kj + 1) * TK],
                            rhs=qkT[row:row + 64, nt, w * TQ + cs:(w + 1) * TQ],
                            start=True, stop=True)
                    ex = expp.tile([128, 2, TQ], BF16, tag="ex",
                                   name=f"ex{pair}_{w}_{kj}")
                    nc.scalar.activation(
                        out=ex[:, :, cs:], in_=sc[:, :, cs:],
                        func=mybir.ActivationFunctionType.Exp,
                        scale=1.0 / np.sqrt(HD))
                    if d >= 0:
                        # noncausal entries live only in the first TK columns
                        # of the trimmed slice: col_local < key_partition
                        for e in range(2):
                            nc.vector.tensor_mul(
                                out=ex[:, e, cs:cs + TK],
                                in0=ex[:, e, cs:cs + TK], in1=tri_sb)
                    exs[kj] = ex

                def av_unit(kj):
                    cs = max(kj * TK - w * TQ, 0)
                    ex = exs[kj]
                    for e, h in enumerate((h0, h1)):
                        nc.tensor.matmul(
                            av[e][:, cs:], lhsT=v_sb[:, kj, h, :],
                            rhs=ex[:, e, cs:],
                            start=(kj == 0), stop=(kj == kmax - 1))
                    if kj == kmax - 1:
                        for e, h in enumerate((h0, h1)):
                            nc.vector.tensor_copy(
                                out=outhT[h][:, tsl], in_=av[e])

                units = []
                for kj in range(kmax):
                    units.append(lambda kj=kj: sc_unit(kj))
                    if kj >= LAG:
                        units.append(lambda kj=kj: av_unit(kj - LAG))
                for kj in range(max(kmax - LAG, 0), kmax):
                    units.append(lambda kj=kj: av_unit(kj))
                return units

            def emit_C(w, heads):
                tsl = slice(w * TQ, (w + 1) * TQ)
                units = []

                def c_unit(h):
                    # bc borrows the fast-cycling "mm" psum slots (a
                    # dedicated tag would exceed the 8-bank budget, and
                    # borrowing av slots can deadlock the static PE order
                    # against an in-flight accumulation)
                    bc = mmps.tile([64, TQ], F32, tag="mm", name=f"bc{h}_{w}")
                    nc.tensor.matmul(bc, lhsT=ones_sb[64:65, :],
                                     rhs=outhT[h][HD:HD + 1, tsl],
                                     start=True, stop=True)
                    rc = dpool.tile([64, TQ], BF16, tag="rc", name=f"rc{h}_{w}")
                    nc.vector.reciprocal(out=rc, in_=bc)
                    if h % 2 == 0:
                        nc.vector.tensor_mul(
                            out=pair_sb[h // 2][0:64, tsl],
                            in0=outhT[h][0:HD, tsl], in1=rc)
                    else:
                        tmp = dpool.tile([64, TQ], BF16, tag="shift",
                                         name=f"sh{h}_{w}")
                        nc.vector.tensor_mul(
                            out=tmp, in0=outhT[h][0:HD, tsl], in1=rc)
                        nc.sync.dma_start(
                            out=pair_sb[h // 2][64:128, tsl], in_=tmp)

                for h in heads:
                    units.append(lambda h=h: c_unit(h))
                return units

            def emit_D(w):
                units = []

                state = {}

                def d_unit(sub):
                    qt = w * (TQ // TK) + sub
                    if sub == 0:
                        state["ob"] = osb.tile([128, TQ // TK, 1024], BF16,
                                               tag="ob", name=f"ob{w}")
                    po = scps.tile([128, 2, 512], F32, tag="sc", name=f"po{qt}")
                    for p in range(2):
                        for ct in range(2):
                            nc.tensor.matmul(
                                po[:, ct, :],
                                lhsT=pair_sb[p][:, qt * 128:(qt + 1) * 128],
                                rhs=wout_sb[:, p, ct * 512:(ct + 1) * 512],
                                start=(p == 0), stop=(p == 1))
                    ob = state["ob"]
                    if w == NQ - 1:
                        nc.scalar.copy(out=ob[:, sub, :],
                                       in_=po.rearrange("p a b -> p (a b)"))
                    else:
                        nc.vector.tensor_copy(
                            out=ob[:, sub, :], in_=po.rearrange("p a b -> p (a b)"))
                    if sub == TQ // TK - 1:
                        nc.sync.dma_start(
                            out=out[w * TQ:(w + 1) * TQ, :].rearrange(
                                "(s p) c -> p s c", p=128),
                            in_=ob)

                for sub in range(TQ // TK):
                    units.append(lambda sub=sub: d_unit(sub))
                return units

            def interleave(primary, extra):
                """Spread extra units evenly among primary units."""
                out, ei = [], 0
                n, m = len(primary), len(extra)
                for i, u in enumerate(primary):
                    out.append(u)
                    while m and ei < m and ei * n < m * (i + 1):
                        out.append(extra[ei])
                        ei += 1
                out.extend(extra[ei:])
                return out

            # Software pipeline: B(w) runs interleaved with the previous
            # window's divide/out-proj and the next window's projections,
            # so the PE has matmul work while ScalarE chews on exps.
            # C for a pair starts as soon as that pair's AV is drained.
            # reps>1 repeats the whole program for wall-clock calibration.
            import contextlib
            loop_cm = (tc.For_i(0, loop_reps, 1, staggered_reset=True)
                       if loop_reps else contextlib.nullcontext())
            with loop_cm:
              for _rep in range(reps):
                hasB = "B" in phases
                hasC = "C" in phases and hasB
                hasD = "D" in phases and hasC
                for u in emit_A(0):
                    u()
                for w in range(NQ):
                    if w >= 1 and loop_reps:
                        # staggered-reset stage boundary: one window per
                        # stage, so consecutive loop reps overlap by up to
                        # two windows instead of a full-barrier back-edge
                        tc.stage_boundary()
                    b0 = emit_B(w, 0) if hasB else []
                    extra0 = (emit_C(w - 1, (2, 3)) + (emit_D(w - 1) if hasD else [])) if (w >= 1 and hasC) else []
                    for u in interleave(b0, extra0):
                        u()
                    b1 = emit_B(w, 1) if hasB else []
                    extra1 = emit_C(w, (0, 1)) if hasC else []
                    if w + 1 < NQ:
                        extra1 += emit_A(w + 1)
                    for u in interleave(b1, extra1):
                        u()
                if hasC:
                    for u in emit_C(NQ - 1, (2, 3)) + (emit_D(NQ - 1) if hasD else []):
                        u()

    if legalize:
        _legalize_single_wait(nc)
    return nc


_NC_CACHE = None


def _get_nc():
    global _NC_CACHE
    if _NC_CACHE is None:
        _NC_CACHE = build_nc()
    return _NC_CACHE


BF = ml_dtypes.bfloat16


def _shard_inputs(x, W_qkv, W_out):
    tri = np.ascontiguousarray(
        np.triu(np.ones((TK, TK), dtype=np.float32))).astype(BF)
    in_maps = []
    for b in range(B):
        xTb = np.ascontiguousarray(x[b].T).astype(BF)
        for g in range(4):
            heads = list(range(G * g, G * g + G))
            qcols = np.concatenate(
                [W_qkv[:, h * HD:(h + 1) * HD] for h in heads], axis=1)
            kcols = np.concatenate(
                [W_qkv[:, C + h * HD:C + (h + 1) * HD] for h in heads], axis=1)
            wqk = np.ascontiguousarray(
                np.concatenate([qcols, kcols], axis=1)).astype(BF)
            wv = np.ascontiguousarray(
                W_qkv[:, 2 * C + G * g * HD:2 * C + (G * g + G) * HD]).astype(BF)
            wout = np.ascontiguousarray(
                W_out[G * g * HD:(G * g + G) * HD, :]).astype(BF)
            in_maps.append({"xT": xTb, "wqk": wqk, "wv": wv, "wout": wout,
                            "trimask": tri})
    return in_maps


def kernel(x, W_qkv, W_out):
    x = np.asarray(x, dtype=np.float32)
    W_qkv = np.asarray(W_qkv, dtype=np.float32)
    W_out = np.asarray(W_out, dtype=np.float32)
    nc = _get_nc()
    in_maps = _shard_inputs(x, W_qkv, W_out)
    res = run_bass_kernel_spmd(nc, in_maps, list(range(N_CORES))).results
    out = np.zeros((B, T, C), dtype=np.float32)
    for b in range(B):
        acc = np.zeros((T, C), dtype=np.float64)
        for g in range(4):
            acc += res[4 * b + g]["out"].astype(np.float64)
        out[b] = acc.astype(np.float32)
    return out


# revision 14
# speedup vs baseline: 1.0269x; 1.0062x over previous
"""Causal self-attention (B=2, T=2048, C=1024, H=16, hd=64) on 8 TRN2 cores.

Sharding: 2-way batch x 4-way head-group (4 heads/core). Each core
computes qkv for its heads, causal attention, and a row-parallel partial
of the out-projection; the host sums the 4 partials per batch element.

v2: all matmul operands bf16 (host-converted inputs), exact causal
column starts (no 256-col floor), causal mask via DVE multiply with a
precomputed 128x128 triangle tile (gpsimd freed for weight DMA), and
AV matmuls emitted LAG score-blocks behind their scores so the in-order
PE queue never stalls on ScalarE's exp latency.

Device schedule (per core, software-pipelined over 4 query windows):
  A(w): qk^T window (wqk^T x^T -> [512 rows, 512 t]) + v blocks
  B(w,pair): per kj block: S^T = k^T q (two 64-row matmuls, PE row-tile
        concurrent), exp (ScalarE, exact causal-shrunk columns), tri
        mask (DVE mul, diag blocks only), then LAG blocks later
        AV += v_aug^T expS^T (ones column gives softmax denom, row 64)
  C(w): denom broadcast (K=1 matmul), reciprocal, scale; odd heads
        DMA-shifted to partitions 64-127 of the pair tile
  D(w): out partial [512, 1024] = sum_pairs pair^T W_out rows (K=128)
"""
import numpy as np
import ml_dtypes

import concourse.bass as bass
import concourse.tile as tile
from concourse import mybir
from concourse.bass_utils import run_bass_kernel_spmd

B, T, C = 2, 2048, 1024
H, HD = 16, 64
G = 4            # heads per core
N_CORES = 8
F32 = mybir.dt.float32
BF16 = mybir.dt.bfloat16
TQ = 512         # query window (moving free dim)
TK = 128         # key block (psum partition)
NQ = T // TQ     # 4 query windows
NK = T // TK     # 16 key blocks
CCH = C // 128   # 8 contraction chunks
LAG = 5          # AV trails scores by LAG blocks (covers exp latency)


def _legalize_single_wait(nc):
    """This walrus build rejects >1 sync wait per instruction. Split
    multi-wait instructions into single-wait NoOp carriers on the same
    engine queue (in-order execution keeps semantics identical)."""
    n = 0
    for f in nc.m.functions:
        for blk in f.blocks:
            insts = list(blk.instructions)
            new = []
            changed = False
            for inst in insts:
                si = inst.sync_info
                if si is not None and len(si.on_wait) > 1:
                    waits = list(si.on_wait)
                    for j, w in enumerate(waits[:-1]):
                        new.append(mybir.InstNoOp(
                            name=f"{inst.name}-sw{j}",
                            engine=inst.engine,
                            sync_info=mybir.SyncInfo(on_wait=[w], on_update=[]),
                            bass_nofuse=True,
                        ))
                    inst.sync_info = mybir.SyncInfo(
                        on_wait=[waits[-1]], on_update=list(si.on_update))
                    changed = True
                    n += 1
                new.append(inst)
            if changed:
                blk.instructions = new
    return n


def build_nc(legalize=True, reps=1, loop_reps=None, phases="ABCD"):
    nc = bass.Bass("TRN2", target_bir_lowering=False, debug=False,
                   num_devices=N_CORES)
    xT = nc.dram_tensor("xT", [C, T], BF16, kind="ExternalInput").ap()
    wqk = nc.dram_tensor("wqk", [C, 2 * G * HD], BF16, kind="ExternalInput").ap()
    wv = nc.dram_tensor("wv", [C, G * HD], BF16, kind="ExternalInput").ap()
    wout = nc.dram_tensor("wout", [G * HD, C], BF16, kind="ExternalInput").ap()
    trimask = nc.dram_tensor("trimask", [TK, TK], BF16, kind="ExternalInput").ap()
    out = nc.dram_tensor("out", [T, C], BF16, kind="ExternalOutput").ap()

    with tile.TileContext(nc) as tc, \
         nc.allow_low_precision(reason="bf16 kernel, 2e-2 rel-err budget"):
        with tc.tile_pool(name="weights", bufs=1) as wpool, \
             tc.tile_pool(name="qkT", bufs=1) as qkpool, \
             tc.tile_pool(name="vsb", bufs=1) as vpool, \
             tc.tile_pool(name="outh", bufs=1) as opool, \
             tc.tile_pool(name="pairs", bufs=1) as ppool, \
             tc.tile_pool(name="xt", bufs=2) as xtp, \
             tc.tile_pool(name="expS", bufs=LAG + 3) as expp, \
             tc.tile_pool(name="div", bufs=3) as dpool, \
             tc.tile_pool(name="osb", bufs=1) as osb, \
             tc.tile_pool(name="mm_ps", bufs=2, space="PSUM") as mmps, \
             tc.tile_pool(name="sc_ps", bufs=2, space="PSUM") as scps, \
             tc.tile_pool(name="av_ps", bufs=1, space="PSUM") as avps:

            wqk_sb = wpool.tile([128, CCH, 512], BF16)
            wv_sb = wpool.tile([128, CCH, 256], BF16)
            wout_sb = wpool.tile([128, 2, 1024], BF16)
            # lhsT for the K=1 denominator-broadcast matmul must share the
            # rhs's base partition (64 = outhT's denominator row)
            ones_sb = wpool.tile([128, 64], BF16)
            tri_sb = wpool.tile([TK, TK], BF16)
            # weights stream on the (initially idle) gpsimd queue, split in
            # chunks so the first matmuls start while the rest arrives; xt
            # loads use the sync queue in parallel
            nc.gpsimd.dma_start(out=tri_sb, in_=trimask)
            for half in range(2):
                nc.gpsimd.dma_start(
                    out=wqk_sb[:, 4 * half:4 * half + 4, :],
                    in_=wqk.rearrange("(c p) n -> p c n", p=128)
                        [:, 4 * half:4 * half + 4, :])
            nc.gpsimd.dma_start(
                out=wv_sb, in_=wv.rearrange("(c p) n -> p c n", p=128))
            nc.gpsimd.dma_start(
                out=wout_sb, in_=wout.rearrange("(c p) n -> p c n", p=128))
            nc.vector.memset(ones_sb, 1.0)

            # qkT[:, nt, :]: nt 0-1 = q^T of heads (0,1),(2,3); nt 2-3 = k^T
            qkT = qkpool.tile([128, 4, T], BF16)
            # v_sb[:, kj, h, 0:64] = v, [..., 64] = 1.0 (only the ones
            # column is memset; the rest is overwritten by A-phase copies)
            v_sb = vpool.tile([128, NK, G, HD + 1], BF16)
            nc.vector.memset(v_sb[:, :, :, HD:HD + 1], 1.0)
            # outhT[h]: [65, T]; rows 0-63 = (expS @ v)^T, row 64 = denom
            outhT = [opool.tile([HD + 1, T], BF16, tag=f"outh{h}", name=f"outhT{h}")
                     for h in range(G)]
            # pair_sb[p]: [128, T] = scaled head 2p (rows 0-63) / 2p+1 (64-127)
            pair_sb = [ppool.tile([128, T], BF16, tag=f"pair{p}", name=f"pair{p}")
                       for p in range(2)]

            def emit_A(w):
                """qk^T window + v blocks, as one unit per matmul group."""
                tsl = slice(w * TQ, (w + 1) * TQ)
                xt = xtp.tile([128, CCH, TQ], BF16, name=f"xt{w}", tag="xt")
                for half in range(2):
                    nc.sync.dma_start(
                        out=xt[:, 4 * half:4 * half + 4, :],
                        in_=xT.rearrange("(c p) t -> p c t", p=128)
                            [:, 4 * half:4 * half + 4, tsl])
                units = []

                def qk_unit(nt):
                    ps = mmps.tile([128, TQ], F32, tag="mm", name=f"qk{w}_{nt}")
                    for cc in range(CCH):
                        nc.tensor.matmul(
                            ps, lhsT=wqk_sb[:, cc, nt * 128:(nt + 1) * 128],
                            rhs=xt[:, cc, :], start=(cc == 0), stop=(cc == CCH - 1))
                    nc.vector.tensor_copy(out=qkT[:, nt, tsl], in_=ps)

                def v_unit(sub):
                    kj = w * (TQ // TK) + sub
                    psv = mmps.tile([128, G * HD], F32, tag="mm", name=f"v{w}_{sub}")
                    for cc in range(CCH):
                        nc.tensor.matmul(
                            psv, lhsT=xt[:, cc, sub * TK:(sub + 1) * TK],
                            rhs=wv_sb[:, cc, :], start=(cc == 0), stop=(cc == CCH - 1))
                    nc.vector.tensor_copy(
                        out=v_sb[:, kj, :, 0:HD],
                        in_=psv.rearrange("p (h d) -> p h d", h=G))

                for nt in range(4):
                    units.append(lambda nt=nt: qk_unit(nt))
                for sub in range(TQ // TK):
                    units.append(lambda sub=sub: v_unit(sub))
                return units

            def emit_B(w, pair):
                """Scores + exp + mask per kj; AV lagged LAG blocks behind."""
                tsl = slice(w * TQ, (w + 1) * TQ)
                kmax = (w + 1) * (TQ // TK)
                h0, h1 = 2 * pair, 2 * pair + 1
                av = [avps.tile([HD + 1, TQ], F32, tag=f"av{e}",
                                name=f"av{pair}_{w}_{e}") for e in range(2)]
                exs = {}

                def sc_unit(kj):
                    d = kj * TK - w * TQ      # >=0 on diagonal blocks
                    cs = max(d, 0)
                    sc = scps.tile([128, 2, TQ], F32, tag="sc",
                                   name=f"sc{pair}_{w}_{kj}")
                    for e, h in enumerate((h0, h1)):
                        row = 64 * (h % 2)
                        nt = h // 2
                        nc.tensor.matmul(
                            sc[:, e, cs:],
                            lhsT=qkT[row:row + 64, 2 + nt, kj * TK:(kj + 1) * TK],
                            rhs=qkT[row:row + 64, nt, w * TQ + cs:(w + 1) * TQ],
                            start=True, stop=True)
                    ex = expp.tile([128, 2, TQ], BF16, tag="ex",
                                   name=f"ex{pair}_{w}_{kj}")
                    nc.scalar.activation(
                        out=ex[:, :, cs:], in_=sc[:, :, cs:],
                        func=mybir.ActivationFunctionType.Exp,
                        scale=1.0 / np.sqrt(HD))
                    if d >= 0:
                        # noncausal entries live only in the first TK columns
                        # of the trimmed slice: col_local < key_partition
                        for e in range(2):
                            nc.vector.tensor_mul(
                                out=ex[:, e, cs:cs + TK],
                                in0=ex[:, e, cs:cs + TK], in1=tri_sb)
                    exs[kj] = ex

                def av_unit(kj):
                    cs = max(kj * TK - w * TQ, 0)
                    ex = exs[kj]
                    for e, h in enumerate((h0, h1)):
                        nc.tensor.matmul(
                            av[e][:, cs:], lhsT=v_sb[:, kj, h, :],
                            rhs=ex[:, e, cs:],
                            start=(kj == 0), stop=(kj == kmax - 1))
                    if kj == kmax - 1:
                        for e, h in enumerate((h0, h1)):
                            nc.vector.tensor_copy(
                                out=outhT[h][:, tsl], in_=av[e])

                units = []
                for kj in range(kmax):
                    units.append(lambda kj=kj: sc_unit(kj))
                    if kj >= LAG:
                        units.append(lambda kj=kj: av_unit(kj - LAG))
                for kj in range(max(kmax - LAG, 0), kmax):
                    units.append(lambda kj=kj: av_unit(kj))
                return units

            def emit_C(w, heads):
                tsl = slice(w * TQ, (w + 1) * TQ)
                units = []

                def c_unit(h):
                    # bc borrows the fast-cycling "mm" psum slots (a
                    # dedicated tag would exceed the 8-bank budget, and
                    # borrowing av slots can deadlock the static PE order
                    # against an in-flight accumulation)
                    bc = mmps.tile([64, TQ], F32, tag="mm", name=f"bc{h}_{w}")
                    nc.tensor.matmul(bc, lhsT=ones_sb[64:65, :],
                                     rhs=outhT[h][HD:HD + 1, tsl],
                                     start=True, stop=True)
                    rc = dpool.tile([64, TQ], BF16, tag="rc", name=f"rc{h}_{w}")
                    nc.vector.reciprocal(out=rc, in_=bc)
                    if h % 2 == 0:
                        nc.vector.tensor_mul(
                            out=pair_sb[h // 2][0:64, tsl],
                            in0=outhT[h][0:HD, tsl], in1=rc)
                    else:
                        tmp = dpool.tile([64, TQ], BF16, tag="shift",
                                         name=f"sh{h}_{w}")
                        nc.vector.tensor_mul(
                            out=tmp, in0=outhT[h][0:HD, tsl], in1=rc)
                        nc.sync.dma_start(
                            out=pair_sb[h // 2][64:128, tsl], in_=tmp)

                for h in heads:
                    units.append(lambda h=h: c_unit(h))
                return units

            def emit_D(w):
                units = []

                state = {}

                def d_unit(sub):
                    qt = w * (TQ // TK) + sub
                    if sub == 0:
                        state["ob"] = osb.tile([128, TQ // TK, 1024], BF16,
                                               tag="ob", name=f"ob{w}")
                    po = scps.tile([128, 2, 512], F32, tag="sc", name=f"po{qt}")
                    for p in range(2):
                        for ct in range(2):
                            nc.tensor.matmul(
                                po[:, ct, :],
                                lhsT=pair_sb[p][:, qt * 128:(qt + 1) * 128],
                                rhs=wout_sb[:, p, ct * 512:(ct + 1) * 512],
                                start=(p == 0), stop=(p == 1))
                    ob = state["ob"]
                    if w == NQ - 1:
                        nc.scalar.copy(out=ob[:, sub, :],
                                       in_=po.rearrange("p a b -> p (a b)"))
                    else:
                        nc.vector.tensor_copy(
                            out=ob[:, sub, :], in_=po.rearrange("p a b -> p (a b)"))
                    if sub == TQ // TK - 1:
                        nc.sync.dma_start(
                            out=out[w * TQ:(w + 1) * TQ, :].rearrange(
                                "(s p) c -> p s c", p=128),
                            in_=ob)

                for sub in range(TQ // TK):
                    units.append(lambda sub=sub: d_unit(sub))
                return units

            def interleave(primary, extra):
                """Spread extra units evenly among primary units."""
                out, ei = [], 0
                n, m = len(primary), len(extra)
                for i, u in enumerate(primary):
                    out.append(u)
                    while m and ei < m and ei * n < m * (i + 1):
                        out.append(extra[ei])
                        ei += 1
                out.extend(extra[ei:])
                return out

            # Software pipeline: B(w) runs interleaved with the previous
            # window's divide/out-proj and the next window's projections,
            # so the PE has matmul work while ScalarE chews on exps.
            # C for a pair starts as soon as that pair's AV is drained.
            # reps>1 repeats the whole program for wall-clock calibration.
            import contextlib
            loop_cm = (tc.For_i(0, loop_reps, 1, staggered_reset=True)
                       if loop_reps else contextlib.nullcontext())
            with loop_cm:
              for _rep in range(reps):
                hasB = "B" in phases
                hasC = "C" in phases and hasB
                hasD = "D" in phases and hasC
                for u in emit_A(0):
                    u()
                for w in range(NQ):
                    if w >= 1 and loop_reps:
                        # staggered-reset stage boundary: one window per
                        # stage, so consecutive loop reps overlap by up to
                        # two windows instead of a full-barrier back-edge
                        tc.stage_boundary()
                    b0 = emit_B(w, 0) if hasB else []
                    extra0 = (emit_C(w - 1, (2, 3)) + (emit_D(w - 1) if hasD else [])) if (w >= 1 and hasC) else []
                    for u in interleave(b0, extra0):
                        u()
                    b1 = emit_B(w, 1) if hasB else []
                    extra1 = emit_C(w, (0, 1)) if hasC else []
                    if w + 1 < NQ:
                        extra1 += emit_A(w + 1)
                    for u in interleave(b1, extra1):
                        u()
                if hasC:
                    for u in emit_C(NQ - 1, (2, 3)) + (emit_D(NQ - 1) if hasD else []):
                        u()

    if legalize:
        _legalize_single_wait(nc)
    return nc


_NC_CACHE = None


def _get_nc():
    global _NC_CACHE
    if _NC_CACHE is None:
        _NC_CACHE = build_nc()
    return _NC_CACHE


BF = ml_dtypes.bfloat16


def _shard_inputs(x, W_qkv, W_out):
    tri = np.ascontiguousarray(
        np.triu(np.ones((TK, TK), dtype=np.float32))).astype(BF)
    in_maps = []
    for b in range(B):
        xTb = np.ascontiguousarray(x[b].T).astype(BF)
        for g in range(4):
            heads = list(range(G * g, G * g + G))
            qcols = np.concatenate(
                [W_qkv[:, h * HD:(h + 1) * HD] for h in heads], axis=1)
            kcols = np.concatenate(
                [W_qkv[:, C + h * HD:C + (h + 1) * HD] for h in heads], axis=1)
            wqk = np.ascontiguousarray(
                np.concatenate([qcols, kcols], axis=1)).astype(BF)
            wv = np.ascontiguousarray(
                W_qkv[:, 2 * C + G * g * HD:2 * C + (G * g + G) * HD]).astype(BF)
            wout = np.ascontiguousarray(
                W_out[G * g * HD:(G * g + G) * HD, :]).astype(BF)
            in_maps.append({"xT": xTb, "wqk": wqk, "wv": wv, "wout": wout,
                            "trimask": tri})
    return in_maps


def kernel(x, W_qkv, W_out):
    x = np.asarray(x, dtype=np.float32)
    W_qkv = np.asarray(W_qkv, dtype=np.float32)
    W_out = np.asarray(W_out, dtype=np.float32)
    nc = _get_nc()
    in_maps = _shard_inputs(x, W_qkv, W_out)
    res = run_bass_kernel_spmd(nc, in_maps, list(range(N_CORES))).results
    out = np.zeros((B, T, C), dtype=np.float32)
    for b in range(B):
        acc = np.zeros((T, C), dtype=np.float64)
        for g in range(4):
            acc += res[4 * b + g]["out"].astype(np.float64)
        out[b] = acc.astype(np.float32)
    return out


# revision 18
# speedup vs baseline: 1.0324x; 1.0053x over previous
"""Causal self-attention (B=2, T=2048, C=1024, H=16, hd=64) on 8 TRN2 cores.

Sharding: 2-way batch x 4-way head-group (4 heads/core). Each core
computes qkv for its heads, causal attention, and a row-parallel partial
of the out-projection; the host sums the 4 partials per batch element.

v4: all matmul operands bf16 (host-converted inputs); both head-pairs
interleaved per key block so ONE exp instruction covers all 4 heads
(ScalarE -19us); AV column-packed via PE col-tiling (e0 -> psum rows
0-63, e1 -> 64-127, concurrent); softmax denominators via 4-way
col-packed K=128 ones-matmuls into one shared bank (concurrent);
normalization multiplies the AV psum directly into the pair tile
(no per-head drains, no SBUF->SBUF shift DMAs). AV/den matmuls are
emitted LAG blocks behind their scores so the in-order PE queue never
stalls on ScalarE's exp latency. PSUM: sc 4 + av 2 + den 1 + mm 1 = 8.

Device schedule (per core, software-pipelined over 4 query windows):
  A(w): qk^T window (wqk^T x^T -> [512 rows, 512 t]) + v blocks
  B(w): per kj block: S^T = k^T q (4x 64-row matmuls, PE row-tile
        concurrent per pair), one exp [128,4,nv] (ScalarE), tri mask
        (one DVE mul, diag blocks only); LAG blocks later AV (col-
        packed) and den (4-way packed) accumulate over kj
  C(w): per pair: 2 col-packed K=1 denom-broadcast matmuls, DVE
        reciprocal, one [128,512] mul av_psum*rc -> pair tile
  D(w): out partial [512, 1024] = sum_pairs pair^T W_out rows (K=128)
"""
import numpy as np
import ml_dtypes

import concourse.bass as bass
import concourse.tile as tile
from concourse import mybir
from concourse.bass_utils import run_bass_kernel_spmd

B, T, C = 2, 2048, 1024
H, HD = 16, 64
G = 4            # heads per core
N_CORES = 8
F32 = mybir.dt.float32
BF16 = mybir.dt.bfloat16
TQ = 512         # query window (moving free dim)
TK = 128         # key block (psum partition)
NQ = T // TQ     # 4 query windows
NK = T // TK     # 16 key blocks
CCH = C // 128   # 8 contraction chunks
LAG = 5          # AV/den trail scores by LAG blocks (covers exp latency)


def _legalize_single_wait(nc):
    """This walrus build rejects >1 sync wait per instruction. Split
    multi-wait instructions into single-wait NoOp carriers on the same
    engine queue (in-order execution keeps semantics identical)."""
    n = 0
    for f in nc.m.functions:
        for blk in f.blocks:
            insts = list(blk.instructions)
            new = []
            changed = False
            for inst in insts:
                si = inst.sync_info
                if si is not None and len(si.on_wait) > 1:
                    waits = list(si.on_wait)
                    for j, w in enumerate(waits[:-1]):
                        new.append(mybir.InstNoOp(
                            name=f"{inst.name}-sw{j}",
                            engine=inst.engine,
                            sync_info=mybir.SyncInfo(on_wait=[w], on_update=[]),
                            bass_nofuse=True,
                        ))
                    inst.sync_info = mybir.SyncInfo(
                        on_wait=[waits[-1]], on_update=list(si.on_update))
                    changed = True
                    n += 1
                new.append(inst)
            if changed:
                blk.instructions = new
    return n


def build_nc(legalize=True, reps=1, loop_reps=None, phases="ABCD"):
    nc = bass.Bass("TRN2", target_bir_lowering=False, debug=False,
                   num_devices=N_CORES)
    xT = nc.dram_tensor("xT", [C, T], BF16, kind="ExternalInput").ap()
    wqk = nc.dram_tensor("wqk", [C, 2 * G * HD], BF16, kind="ExternalInput").ap()
    wv = nc.dram_tensor("wv", [C, G * HD], BF16, kind="ExternalInput").ap()
    wout = nc.dram_tensor("wout", [G * HD, C], BF16, kind="ExternalInput").ap()
    trimask = nc.dram_tensor("trimask", [TK, TK], BF16, kind="ExternalInput").ap()
    out = nc.dram_tensor("out", [T, C], BF16, kind="ExternalOutput").ap()

    with tile.TileContext(nc) as tc, \
         nc.allow_low_precision(reason="bf16 kernel, 2e-2 rel-err budget"):
        with tc.tile_pool(name="weights", bufs=1) as wpool, \
             tc.tile_pool(name="qkT", bufs=1) as qkpool, \
             tc.tile_pool(name="vsb", bufs=1) as vpool, \
             tc.tile_pool(name="pairs", bufs=1) as ppool, \
             tc.tile_pool(name="xt", bufs=2) as xtp, \
             tc.tile_pool(name="expS", bufs=LAG + 3) as expp, \
             tc.tile_pool(name="div", bufs=2) as dpool, \
             tc.tile_pool(name="osb", bufs=1) as osb, \
             tc.tile_pool(name="mm_ps", bufs=1, space="PSUM") as mmps, \
             tc.tile_pool(name="sc_ps", bufs=1, space="PSUM") as scps, \
             tc.tile_pool(name="av_ps", bufs=1, space="PSUM") as avps, \
             tc.tile_pool(name="den_ps", bufs=1, space="PSUM") as denps:

            wqk_sb = wpool.tile([128, CCH, 512], BF16)
            wv_sb = wpool.tile([128, CCH, 256], BF16)
            wout_sb = wpool.tile([128, 2, 1024], BF16)
            # ones: full-partition 1.0; column 0 is the den-matmul lhsT,
            # rows {0,32,64,96} are the K=1 denom-broadcast lhsTs
            ones_sb = wpool.tile([128, 64], BF16)
            # triangle mask replicated 4x so one DVE mul covers all heads
            tri4_sb = wpool.tile([TK, G, TK], BF16)
            for i in range(G):
                nc.gpsimd.dma_start(out=tri4_sb[:, i, :], in_=trimask)
            for half in range(2):
                nc.gpsimd.dma_start(
                    out=wqk_sb[:, 4 * half:4 * half + 4, :],
                    in_=wqk.rearrange("(c p) n -> p c n", p=128)
                        [:, 4 * half:4 * half + 4, :])
            nc.gpsimd.dma_start(
                out=wv_sb, in_=wv.rearrange("(c p) n -> p c n", p=128))
            nc.gpsimd.dma_start(
                out=wout_sb, in_=wout.rearrange("(c p) n -> p c n", p=128))
            nc.vector.memset(ones_sb, 1.0)

            # qkT[:, nt, :]: nt 0-1 = q^T of heads (0,1),(2,3); nt 2-3 = k^T
            qkT = qkpool.tile([128, 4, T], BF16)
            # v_sb[:, kj, h, :] = v block (no ones column in v4)
            v_sb = vpool.tile([128, NK, G, HD], BF16)
            # pair_sb[p]: [128, T] = scaled head 2p (rows 0-63) / 2p+1 (64-127)
            pair_sb = [ppool.tile([128, T], BF16, tag=f"pair{p}", name=f"pair{p}")
                       for p in range(2)]

            def emit_A(w):
                """qk^T window + v blocks, as one unit per matmul group."""
                tsl = slice(w * TQ, (w + 1) * TQ)
                xt = xtp.tile([128, CCH, TQ], BF16, name=f"xt{w}", tag="xt")
                for half in range(2):
                    nc.sync.dma_start(
                        out=xt[:, 4 * half:4 * half + 4, :],
                        in_=xT.rearrange("(c p) t -> p c t", p=128)
                            [:, 4 * half:4 * half + 4, tsl])
                units = []

                def qk_unit(nt):
                    ps = mmps.tile([128, TQ], F32, tag="mm", name=f"qk{w}_{nt}")
                    for cc in range(CCH):
                        nc.tensor.matmul(
                            ps, lhsT=wqk_sb[:, cc, nt * 128:(nt + 1) * 128],
                            rhs=xt[:, cc, :], start=(cc == 0), stop=(cc == CCH - 1))
                    nc.vector.tensor_copy(out=qkT[:, nt, tsl], in_=ps)

                def v_unit(sub):
                    kj = w * (TQ // TK) + sub
                    psv = mmps.tile([128, G * HD], F32, tag="mm", name=f"v{w}_{sub}")
                    for cc in range(CCH):
                        nc.tensor.matmul(
                            psv, lhsT=xt[:, cc, sub * TK:(sub + 1) * TK],
                            rhs=wv_sb[:, cc, :], start=(cc == 0), stop=(cc == CCH - 1))
                    nc.vector.tensor_copy(
                        out=v_sb[:, kj, :, :],
                        in_=psv.rearrange("p (h d) -> p h d", h=G))

                for nt in range(4):
                    units.append(lambda nt=nt: qk_unit(nt))
                for sub in range(TQ // TK):
                    units.append(lambda sub=sub: v_unit(sub))
                return units

            def emit_B(w):
                """Both pairs per kj: 4 scores + 1 exp + mask; col-packed
                AV + 4-way-packed den accumulate LAG blocks behind."""
                kmax = (w + 1) * (TQ // TK)
                av = [avps.tile([128, TQ], F32, tag=f"av{p}",
                                name=f"av{w}_{p}") for p in range(2)]
                den = denps.tile([128, TQ], F32, tag="den", name=f"den{w}")
                den_sb = dpool.tile([128, TQ], BF16, tag="densb",
                                    name=f"densb{w}")
                exs = {}

                def sc_unit(kj):
                    d = kj * TK - w * TQ      # >=0 on diagonal blocks
                    cs = max(d, 0)
                    sc = scps.tile([128, G, TQ], F32, tag="sc",
                                   name=f"sc{w}_{kj}")
                    for p in range(2):
                        for e in range(2):
                            nc.tensor.matmul(
                                sc[:, 2 * p + e, cs:],
                                lhsT=qkT[64 * e:64 * e + 64, 2 + p,
                                         kj * TK:(kj + 1) * TK],
                                rhs=qkT[64 * e:64 * e + 64, p,
                                        w * TQ + cs:(w + 1) * TQ],
                                start=True, stop=True)
                    ex = expp.tile([128, G, TQ], BF16, tag="ex",
                                   name=f"ex{w}_{kj}")
                    nc.scalar.activation(
                        out=ex[:, :, cs:], in_=sc[:, :, cs:],
                        func=mybir.ActivationFunctionType.Exp,
                        scale=1.0 / np.sqrt(HD))
                    if d >= 0:
                        # noncausal entries live only in the first TK columns
                        # of the trimmed slice: col_local < key_partition
                        nc.vector.tensor_mul(
                            out=ex[:, :, cs:cs + TK],
                            in0=ex[:, :, cs:cs + TK], in1=tri4_sb)
                    exs[kj] = ex

                def avden_unit(kj):
                    cs = max(kj * TK - w * TQ, 0)
                    ex = exs[kj]
                    st, sp = (kj == 0), (kj == kmax - 1)
                    for p in range(2):
                        for e in range(2):
                            # e0 -> psum rows 0-63, e1 -> 64-127 (col tiles)
                            nc.tensor.matmul(
                                av[p][64 * e:64 * e + 64, cs:],
                                lhsT=v_sb[:, kj, 2 * p + e, :],
                                rhs=ex[:, 2 * p + e, cs:], start=st, stop=sp)
                    for p in range(2):
                        for e in range(2):
                            r = 32 * (2 * p + e)
                            nc.tensor.matmul(
                                den[r:r + 1, cs:], lhsT=ones_sb[:, 0:1],
                                rhs=ex[:, 2 * p + e, cs:], start=st, stop=sp,
                                tile_position=(0, r))
                    if sp:
                        nc.vector.tensor_copy(out=den_sb, in_=den)

                units = []
                for kj in range(kmax):
                    units.append(lambda kj=kj: sc_unit(kj))
                    if kj >= LAG:
                        units.append(lambda kj=kj: avden_unit(kj - LAG))
                for kj in range(max(kmax - LAG, 0), kmax):
                    units.append(lambda kj=kj: avden_unit(kj))
                return units, av, den_sb

            def emit_C(w, av, den_sb):
                tsl = slice(w * TQ, (w + 1) * TQ)
                units = []

                def c_unit(p):
                    bc = mmps.tile([128, TQ], F32, tag="mm", name=f"bc{w}_{p}")
                    for e in range(2):
                        r = 32 * (2 * p + e)
                        nc.tensor.matmul(
                            bc[64 * e:64 * e + 64, :],
                            lhsT=ones_sb[r:r + 1, :], rhs=den_sb[r:r + 1, :],
                            start=True, stop=True, tile_position=(r, 64 * e))
                    rc = dpool.tile([128, TQ], BF16, tag="rc", name=f"rc{w}_{p}")
                    nc.vector.reciprocal(out=rc, in_=bc)
                    nc.vector.tensor_mul(
                        out=pair_sb[p][:, tsl], in0=av[p], in1=rc)

                for p in range(2):
                    units.append(lambda p=p: c_unit(p))
                return units

            def emit_D(w):
                units = []

                state = {}

                def d_unit(sub):
                    qt = w * (TQ // TK) + sub
                    if sub == 0:
                        state["ob"] = osb.tile([128, TQ // TK, 1024], BF16,
                                               tag="ob", name=f"ob{w}")
                    ob = state["ob"]
                    for ct in range(2):
                        po = mmps.tile([128, 512], F32, tag="mm",
                                       name=f"po{qt}_{ct}")
                        for p in range(2):
                            nc.tensor.matmul(
                                po,
                                lhsT=pair_sb[p][:, qt * 128:(qt + 1) * 128],
                                rhs=wout_sb[:, p, ct * 512:(ct + 1) * 512],
                                start=(p == 0), stop=(p == 1))
                        if w == NQ - 1:
                            nc.scalar.copy(out=ob[:, sub, ct * 512:(ct + 1) * 512],
                                           in_=po)
                        else:
                            nc.vector.tensor_copy(
                                out=ob[:, sub, ct * 512:(ct + 1) * 512], in_=po)
                    if sub == TQ // TK - 1:
                        nc.sync.dma_start(
                            out=out[w * TQ:(w + 1) * TQ, :].rearrange(
                                "(s p) c -> p s c", p=128),
                            in_=ob)

                for sub in range(TQ // TK):
                    units.append(lambda sub=sub: d_unit(sub))
                return units

            def interleave(primary, extra):
                """Spread extra units evenly among primary units."""
                out, ei = [], 0
                n, m = len(primary), len(extra)
                for i, u in enumerate(primary):
                    out.append(u)
                    while m and ei < m and ei * n < m * (i + 1):
                        out.append(extra[ei])
                        ei += 1
                out.extend(extra[ei:])
                return out

            # Software pipeline: B(w) runs interleaved with the previous
            # window's divide/out-proj and the next window's projections,
            # so the PE has matmul work while ScalarE chews on exps.
            import contextlib
            loop_cm = (tc.For_i(0, loop_reps, 1, staggered_reset=True)
                       if loop_reps else contextlib.nullcontext())
            with loop_cm:
              for _rep in range(reps):
                hasB = "B" in phases
                hasC = "C" in phases and hasB
                hasD = "D" in phases and hasC
                for u in emit_A(0):
                    u()
                prev = None   # (av, den_sb) of window w-1
                for w in range(NQ):
                    if w >= 1 and loop_reps:
                        # staggered-reset stage boundary: one window per
                        # stage, so consecutive loop reps overlap instead
                        # of a full-barrier back-edge
                        tc.stage_boundary()
                    if hasB:
                        bunits, av, den_sb = emit_B(w)
                    else:
                        bunits, av, den_sb = [], None, None
                    extras = []
                    if prev is not None and hasC:
                        extras += emit_C(w - 1, *prev)
                        if hasD:
                            extras += emit_D(w - 1)
                    if w + 1 < NQ:
                        extras += emit_A(w + 1)
                    for u in interleave(bunits, extras):
                        u()
                    prev = (av, den_sb)
                if hasC and prev is not None:
                    for u in emit_C(NQ - 1, *prev) + (emit_D(NQ - 1) if hasD else []):
                        u()

    if legalize:
        _legalize_single_wait(nc)
    return nc


_NC_CACHE = None


def _get_nc():
    global _NC_CACHE
    if _NC_CACHE is None:
        _NC_CACHE = build_nc()
    return _NC_CACHE


BF = ml_dtypes.bfloat16


def _shard_inputs(x, W_qkv, W_out):
    tri = np.ascontiguousarray(
        np.triu(np.ones((TK, TK), dtype=np.float32))).astype(BF)
    in_maps = []
    for b in range(B):
        xTb = np.ascontiguousarray(x[b].T).astype(BF)
        for g in range(4):
            heads = list(range(G * g, G * g + G))
            qcols = np.concatenate(
                [W_qkv[:, h * HD:(h + 1) * HD] for h in heads], axis=1)
            kcols = np.concatenate(
                [W_qkv[:, C + h * HD:C + (h + 1) * HD] for h in heads], axis=1)
            wqk = np.ascontiguousarray(
                np.concatenate([qcols, kcols], axis=1)).astype(BF)
            wv = np.ascontiguousarray(
                W_qkv[:, 2 * C + G * g * HD:2 * C + (G * g + G) * HD]).astype(BF)
            wout = np.ascontiguousarray(
                W_out[G * g * HD:(G * g + G) * HD, :]).astype(BF)
            in_maps.append({"xT": xTb, "wqk": wqk, "wv": wv, "wout": wout,
                            "trimask": tri})
    return in_maps


def kernel(x, W_qkv, W_out):
    x = np.asarray(x, dtype=np.float32)
    W_qkv = np.asarray(W_qkv, dtype=np.float32)
    W_out = np.asarray(W_out, dtype=np.float32)
    nc = _get_nc()
    in_maps = _shard_inputs(x, W_qkv, W_out)
    res = run_bass_kernel_spmd(nc, in_maps, list(range(N_CORES))).results
    out = np.zeros((B, T, C), dtype=np.float32)
    for b in range(B):
        acc = np.zeros((T, C), dtype=np.float64)
        for g in range(4):
            acc += res[4 * b + g]["out"].astype(np.float64)
        out[b] = acc.astype(np.float32)
    return out
